# revision 27
# baseline (speedup 1.0000x reference)
"""ConvTranspose2d (16,256,32,32) -> (16,128,66,66), stride 2, 4x4 kernel.

Strategy: data-parallel over batch, 2 images per core on 8 NeuronCores.

Math: y[b,co,2m+p,2n+q] = bias[co]
        + sum_{i,j in {0,1}} sum_ci x[b,ci,m-i,n-j] * w[ci,co,p+2i,q+2j]
for parity class (p,q) in {0,1}^2, m,n in [0,33).

Per image and parity class: output subgrid [128co x 33 x 33] computed
in row-chunks; each chunk is one PSUM accumulation group of 8 bf16
matmuls (2 ci-chunks x 4 taps (i,j)), K=128, M=128, N=R*34, accumulated
in fp32 PSUM.  Inputs ride in bf16 and the output ships bf16 (host
upcasts): total rel err ~2.9e-3 vs the 2e-2 budget.  Shifted taps read
a zero-padded 34x34 SBUF copy of x (padded host-side); the pad column
rides along in the matmul free dim and is cropped on drain (a strided
no-pad rhs costs more in PE row-turnaround than the pad column does).

Measured-schedule notes (from NTFF traces of this exact kernel):
- The PE matmul stream runs gap-free at ~2.35GHz effective once HAM
  grants full clock; the stream itself (~31us) is the bf16 roofline.
  The tunable parts are the head (time to first real matmul) and the
  tail (last matmul -> all engines idle; a fixed ~7.5us walrus
  postamble that clears every semaphore follows and cannot be changed).
- Head: the sync HWDGE ring starts moving data ~2.7us into the exec
  window at ~170GB/s; the scalar ring ~3.7us at ~107-170GB/s (2048B
  runs are descriptor-limited).  The first real matmul group is a
  small 5-row chunk needing only w00 + x0 rows 0..6, and its two
  inputs ride the two rings CONCURRENTLY:
    sync:   w00, x0B[0:64], x0C[0:64], w01, x1
    scalar: x0A (rows 0..6), x0B[64:128], x0C[64:128], w(1,*)
  so the first real matmul fires ~6.0us in (vs 7.5 for the naive
  order); every later piece lands just ahead of the group needing it.
- A bf16 matmul warmup on gpsimd-memset garbage bridges the DMA wait
  and absorbs the HAM 1.2GHz cold period (~3.4us); it is sized to end
  just before w00+x0A land.
- Drains fuse bias add, bf16 downcast and parity de-interleave in one
  DVE tensor_scalar_add.  Chunk pairs drain as ONE op reading both
  PSUM banks of a [128,2,512] two-bank tile (4D APs): 16 drains
  instead of 28 -> fewer cross-engine sem edges and a shorter tail.
- Outputs: image-0 assembles in SBUF, leaves as 3 row-band DMAs
  (gpsimd / HW-rings / gpsimd); image-1 is band-major with bands
  (14,11,7,1) parity rows so the band trailing the last matmul is a
  single 264B/partition transfer on the (by then idle) HW rings.
"""

import numpy as np
import ml_dtypes

import concourse.bass as bass
import concourse.bacc as bacc
import concourse.tile as tile
from concourse import mybir
from concourse.bass_utils import run_bass_kernel_spmd

N_CORES = 8
B_PER = 2  # images per core

F32 = mybir.dt.float32
BF16 = mybir.dt.bfloat16

PW = 34            # padded x width (32 + 1 left + 1 right)
XLEN = PW * PW     # 1156 padded x elems per partition
XPAD = 1160        # sbuf/dram x free size (AP slack for the last chunk)

# (m0, R) parity-row chunks; rows m0..m0+R-1 of the 33-row parity grid.
# One PSUM accumulation group per chunk: N = R*34 fp32 <= 2KB PSUM bank.
# Image 0 is [single, pairA, pairB] when SINGLE_FIRST else
# [pairA, pairB, single]; the two pair chunks must have equal R.
CHUNKS0 = [(0, 5), (5, 14), (19, 14)]
SINGLE_FIRST = True
# image 1 drains through progressively smaller output bands so the
# last band's DMA is tiny and almost nothing trails the final matmul
CHUNKS1 = [(0, 14), (14, 9), (23, 8), (31, 2)]
PAIR_BUFS = 3
SINGLE_BUFS = 1

# x0 row-pieces (padded-row element ranges of the 1156-elem plane):
# piece A covers the first chunk's rows, B the second chunk's, C the rest.
X0A = 240
X0B = 714
N_WARMUP = 13
# HEAD2: w00 rides the sync ring whole while x0A rides the scalar ring
# whole (the two first-group inputs land concurrently); x0B/x0C are
# partition-halved across both rings.
HEAD2 = True
# HXFUSE: host packs [x0 rows 0..7 | w00] into one DRAM tensor arriving
# as ONE fat-run sync DMA.  A/B-measured WORSE than HEAD2 (serializing
# 385KB onto one ring loses to the two rings working in parallel on
# w00 and x0A); kept for reference, off by default.
HXFUSE = False
X0MAIN = 170
# TAIL3: split the second-to-last image-1 band across sync/scalar/
# gpsimd.  A/B-measured neutral-to-worse (the gpsimd SWDGE piece
# straggles); off by default.
TAIL3 = False

# partition split across the two HW rings for halved transfers
PSPLIT = 64


def _emit_group(nc, ps, wv, xv, m0, R):
    """One PSUM accumulation group: 8 matmuls for one class, one chunk.
    wv is the class's [ci', c, i, j, co] weight view; xv the [ci', c, e]
    padded-x view.  The rhs stays contiguous (the pad column rides along
    and is cropped on drain)."""
    nf = R * PW
    k = 0
    for c in range(2):
        for i in range(2):
            for j in range(2):
                off = (m0 - i + 1) * PW + (1 - j)
                nc.tensor.matmul(
                    ps,
                    wv[:, c, i, j, :],
                    xv[:, c, off:off + nf],
                    start=(k == 0),
                    stop=(k == 7),
                )
                k += 1


TARGET_BIR = False


def build_nc(debug: bool = False) -> bass.Bass:
    nc = bacc.Bacc("TRN2", target_bir_lowering=TARGET_BIR, debug=debug,
                   num_devices=N_CORES)

    # x arrives host-padded bf16: [img, ci'=128, c=2, 34*34+tail] flat
    x_d = nc.declare_dram_parameter("x", [B_PER, 128, 2, XPAD], BF16,
                                    isOutput=False)
    # w layout: [ci'=128, p, q, c, i, j, co] -- a whole class is one
    # contiguous 2KB run per partition, so one DMA per class is efficient
    w_d = nc.declare_dram_parameter("w", [128, 2, 2, 2, 2, 2, 128], BF16,
                                    isOutput=False)
    b_d = nc.declare_dram_parameter("b", [128, 1], F32, isOutput=False)
    # fused [x0 rows 0..7 | w00] head transfer (see HXFUSE)
    hx_d = nc.declare_dram_parameter("hx", [128, 2 * X0A + 1024], BF16,
                                     isOutput=False)
    # y ships as bf16 (host upcasts): halves output HBM traffic and
    # doubles DVE drain throughput; adds ~2e-3 rounding (budget 2e-2)
    y_d = nc.declare_dram_parameter("y", [B_PER, 128, 66, 66], BF16,
                                    isOutput=True)

    with tile.TileContext(nc) as tc:
        with (
            tc.tile_pool(name="wp", bufs=1) as wpool,
            tc.tile_pool(name="bp", bufs=1) as bpool,
            tc.tile_pool(name="xp", bufs=B_PER) as xpool,
            tc.tile_pool(name="yp", bufs=1) as ypool,
            tc.tile_pool(name="ybp", bufs=4) as bandpool,
            tc.tile_pool(name="wu", bufs=1) as wupool,
            tc.tile_pool(name="hxp", bufs=1) as hxpool,
            tc.tile_pool(name="pd", bufs=PAIR_BUFS, space="PSUM") as pairpool,
            tc.tile_pool(name="psg", bufs=SINGLE_BUFS, space="PSUM") as singlepool,
            tc.tile_pool(name="pw", bufs=1, space="PSUM") as warmpool,
        ):
            # PE warm-up on gpsimd-memset garbage (gpsimd runs user code
            # first, so the warmup starts ~1us earlier than via vector).
            # HAM starts the PE at 1.2GHz and unthrottles after ~3.4us of
            # sustained activity; the dummies bridge until w00+x0A land.
            wub = wupool.tile([128, 512], BF16)
            nc.gpsimd.memset(wub[:], 0.0)
            wps = warmpool.tile([128, 512], F32)
            for _ in range(N_WARMUP):
                nc.tensor.matmul(wps[:], wub[:, 0:128], wub[:],
                                 start=True, stop=True)

            # bias rides gpsimd (the output queue, idle at start) so it
            # lands before the first drain without delaying inputs
            bt = bpool.tile([128, 1], F32)
            nc.gpsimd.dma_start(out=bt[:], in_=b_d[:])

            wt = wpool.tile([128, 2, 2, 2, 2, 2, 128], BF16)
            xt = [xpool.tile([128, 2, XPAD], BF16, name=f"x{img}", tag="xt")
                  for img in range(B_PER)]
            hxt = hxpool.tile([128, 2 * X0A + 1024], BF16)
            hx_x = hxt[:, 0:2 * X0A].rearrange("p (c e) -> p c e", c=2)
            hx_w = hxt[:, 2 * X0A:].rearrange("p (c i j o) -> p c i j o",
                                              c=2, i=2, j=2)

            def class_w(p, q):
                if HXFUSE and p == 0 and q == 0:
                    return hx_w
                return wt[:, p, q]

            # Input schedule (see module docstring): the two first-group
            # inputs (x0 piece A and w00) land as early as possible; the
            # later x0 pieces arrive just ahead of the chunks needing them.
            if HXFUSE:
                nc.sync.dma_start(out=hxt[:], in_=hx_d[:])
                nc.sync.dma_start(out=xt[0][0:PSPLIT, :, X0MAIN:X0B],
                                  in_=x_d[0][0:PSPLIT, :, X0MAIN:X0B])
                nc.scalar.dma_start(out=xt[0][PSPLIT:128, :, X0MAIN:X0B],
                                    in_=x_d[0][PSPLIT:128, :, X0MAIN:X0B])
                nc.scalar.dma_start(out=xt[0][:, :, X0B:XPAD],
                                    in_=x_d[0][:, :, X0B:XPAD])
                nc.sync.dma_start(out=wt[:, 0, 1], in_=w_d[:, 0, 1])
                nc.scalar.dma_start(out=wt[:, 1], in_=w_d[:, 1])
                nc.sync.dma_start(out=xt[1][:], in_=x_d[1])
            elif HEAD2:
                nc.scalar.dma_start(out=xt[0][:, :, 0:X0A],
                                    in_=x_d[0][:, :, 0:X0A])
                nc.sync.dma_start(out=wt[:, 0, 0], in_=w_d[:, 0, 0])
                nc.sync.dma_start(out=xt[0][0:PSPLIT, :, X0A:X0B],
                                  in_=x_d[0][0:PSPLIT, :, X0A:X0B])
                nc.scalar.dma_start(out=xt[0][PSPLIT:128, :, X0A:X0B],
                                    in_=x_d[0][PSPLIT:128, :, X0A:X0B])
                nc.sync.dma_start(out=xt[0][0:PSPLIT, :, X0B:XPAD],
                                  in_=x_d[0][0:PSPLIT, :, X0B:XPAD])
                nc.scalar.dma_start(out=xt[0][PSPLIT:128, :, X0B:XPAD],
                                    in_=x_d[0][PSPLIT:128, :, X0B:XPAD])
                nc.sync.dma_start(out=wt[:, 0, 1], in_=w_d[:, 0, 1])
                nc.scalar.dma_start(out=wt[:, 1], in_=w_d[:, 1])
                nc.sync.dma_start(out=xt[1][:], in_=x_d[1])
            else:
                nc.sync.dma_start(out=xt[0][:, :, 0:X0A],
                                  in_=x_d[0][:, :, 0:X0A])
                nc.sync.dma_start(out=wt[:, 0, 0], in_=w_d[:, 0, 0])
                nc.scalar.dma_start(out=xt[0][:, :, X0A:X0B],
                                    in_=x_d[0][:, :, X0A:X0B])
                nc.scalar.dma_start(out=xt[0][:, :, X0B:XPAD],
                                    in_=x_d[0][:, :, X0B:XPAD])
                nc.sync.dma_start(out=wt[:, 0, 1], in_=w_d[:, 0, 1])
                nc.scalar.dma_start(out=wt[:, 1], in_=w_d[:, 1])
                nc.sync.dma_start(out=xt[1][:], in_=x_d[1])

            def drain(ps_view, out_view):
                nc.vector.tensor_scalar_add(out_view, ps_view, bt[:])

            # ---- image 0: class-major (w classes stream in one at a
            # time).  Chunks 1+2 (equal R=14) accumulate into the two
            # banks of one pair tile and drain as a single DVE op.
            yt = ypool.tile([128, 66, 66], BF16)
            if SINGLE_FIRST:
                (m0s, Rs), (m0a, Ra), (m0b, Rb) = CHUNKS0
            else:
                (m0a, Ra), (m0b, Rb), (m0s, Rs) = CHUNKS0
            assert Ra == Rb and m0b == m0a + Ra
            for p in range(2):
                for q in range(2):
                    pt = pairpool.tile([128, 2, 512], F32)
                    st = singlepool.tile([128, 512], F32)

                    def do_single():
                        xv = hx_x if HXFUSE else xt[0]
                        _emit_group(nc, st[:, 0:Rs * PW], class_w(p, q), xv,
                                    m0s, Rs)
                        drain(
                            st[:, 0:Rs * PW]
                            .rearrange("p (m n) -> p m n", n=PW)[:, :, 0:33],
                            yt[:, p::2, q::2][:, m0s:m0s + Rs, :],
                        )

                    def do_pair():
                        _emit_group(nc, pt[:, 0, 0:Ra * PW], class_w(p, q),
                                    xt[0], m0a, Ra)
                        _emit_group(nc, pt[:, 1, 0:Rb * PW], class_w(p, q),
                                    xt[0], m0b, Rb)
                        # both pair chunks leave in one 2-bank drain
                        drain(
                            pt[:, :, 0:Ra * PW]
                            .rearrange("p b (m n) -> p b m n",
                                       n=PW)[:, :, :, 0:33],
                            yt[:, p::2, q::2][:, m0a:m0a + Ra + Rb, :]
                            .rearrange("p (b m) n -> p b m n", b=2),
                        )

                    if SINGLE_FIRST:
                        do_single()
                        do_pair()
                    else:
                        do_pair()
                        do_single()
            for bi, (m0, R) in enumerate(CHUNKS0):
                rows = slice(2 * m0, 2 * (m0 + R))
                if bi == 1:
                    nc.sync.dma_start(out=y_d[0][0:PSPLIT, rows],
                                      in_=yt[0:PSPLIT, rows])
                    nc.scalar.dma_start(out=y_d[0][PSPLIT:128, rows],
                                        in_=yt[PSPLIT:128, rows])
                else:
                    nc.gpsimd.dma_start(out=y_d[0][:, rows], in_=yt[:, rows])

            # ---- image 1: band-major.  Per band the four classes fill
            # two pair tiles (p=0 -> banks q=0,1 of tile A; p=1 -> tile
            # B) and drain as two DVE ops.  The first two bands ride
            # gpsimd (fast outbound) while y0 drains the HW rings; the
            # small last bands ride the HW rings, idle again by then.
            for bi, (m0, R) in enumerate(CHUNKS1):
                band = bandpool.tile([128, 2 * R, 66], BF16)
                for p in range(2):
                    pt = pairpool.tile([128, 2, 512], F32)
                    for q in range(2):
                        _emit_group(nc, pt[:, q, 0:R * PW], class_w(p, q),
                                    xt[1], m0, R)
                    drain(
                        pt[:, :, 0:R * PW]
                        .rearrange("p b (m n) -> p b m n", n=PW)[:, :, :, 0:33],
                        band[:, p::2, :]
                        .rearrange("p m (n q) -> p q m n", q=2),
                    )
                y_view = y_d[1][:, 2 * m0:2 * (m0 + R), :]
                if bi < 2:
                    nc.gpsimd.dma_start(out=y_view, in_=band[:])
                elif TAIL3 and bi == 2:
                    # 3-way split so the last band's ring DMAs aren't
                    # queued behind this band's bytes
                    nc.sync.dma_start(out=y_view[0:32], in_=band[0:32])
                    nc.scalar.dma_start(out=y_view[32:64], in_=band[32:64])
                    nc.gpsimd.dma_start(out=y_view[64:128], in_=band[64:128])
                else:
                    nc.sync.dma_start(out=y_view[0:PSPLIT], in_=band[0:PSPLIT])
                    nc.scalar.dma_start(out=y_view[PSPLIT:128],
                                        in_=band[PSPLIT:128])

    nc.compile()
    return nc


_nc_cache = None


def _get_nc():
    global _nc_cache
    if _nc_cache is None:
        _nc_cache = build_nc()
    return _nc_cache


def make_in_maps(x: np.ndarray, weight: np.ndarray, bias: np.ndarray):
    # w[ci,co,kh,kw] -> [ci', p, q, c, i, j, co]
    w7 = (
        np.asarray(weight, dtype=np.float32)
        .reshape(2, 128, 128, 2, 2, 2, 2)      # [c, ci', co, i, p, j, q]
        .transpose(1, 4, 6, 0, 3, 5, 2)        # -> [ci', p, q, c, i, j, co]
    )
    w_host = np.ascontiguousarray(w7.astype(ml_dtypes.bfloat16))
    b_host = np.ascontiguousarray(
        np.asarray(bias, dtype=np.float32).reshape(128, 1)
    )
    x = np.asarray(x, dtype=np.float32)
    # host-side zero-pad into the 34x34(+tail) layout, ci split [c, ci']
    # transposed to [ci', c], bf16
    xpad = np.zeros((16, 2, 128, XPAD), dtype=np.float32)
    xpad[:, :, :, :XLEN].reshape(16, 2, 128, PW, PW)[:, :, :, 1:33, 1:33] = (
        x.reshape(16, 2, 128, 32, 32)
    )
    xpad = np.ascontiguousarray(
        xpad.transpose(0, 2, 1, 3).astype(ml_dtypes.bfloat16)
    )
    w00_flat = w_host[:, 0, 0].reshape(128, 1024)
    return [
        {
            "x": xpad[B_PER * i:B_PER * (i + 1)],
            "w": w_host,
            "b": b_host,
            # fused [x0 rows 0..7 | w00] head transfer (one fat run)
            "hx": np.ascontiguousarray(np.concatenate(
                [xpad[B_PER * i][:, :, 0:X0A].reshape(128, 2 * X0A),
                 w00_flat], axis=1)),
        }
        for i in range(N_CORES)
    ]


def kernel(x: np.ndarray, weight: np.ndarray, bias: np.ndarray) -> np.ndarray:
    nc = _get_nc()
    in_maps = make_in_maps(x, weight, bias)
    res = run_bass_kernel_spmd(nc, in_maps, list(range(N_CORES)))
    out = np.concatenate([r["y"] for r in res.results], axis=0)
    return np.ascontiguousarray(out.astype(np.float32))


# revision 30
# speedup vs baseline: 1.1497x; 1.1497x over previous
"""ConvTranspose2d (16,256,32,32) -> (16,128,66,66), stride 2, 4x4 kernel.

Strategy: data-parallel over batch, 2 images per core on 8 NeuronCores.

Math: y[b,co,2m+p,2n+q] = bias[co]
        + sum_{i,j in {0,1}} sum_ci x[b,ci,m-i,n-j] * w[ci,co,p+2i,q+2j]
for parity class (p,q) in {0,1}^2, m,n in [0,33).

Per image and parity class: output subgrid [128co x 33 x 33] computed
in row-chunks; each chunk is one PSUM accumulation group of 8 bf16
matmuls (2 ci-chunks x 4 taps (i,j)), K=128, M=128, N=R*34, accumulated
in fp32 PSUM.  Inputs ride in bf16 and the output ships bf16 (host
upcasts): total rel err ~2.9e-3 vs the 2e-2 budget.  Shifted taps read
a zero-padded 34x34 SBUF copy of x (padded host-side); the pad column
rides along in the matmul free dim and is cropped on drain (a strided
no-pad rhs costs more in PE row-turnaround than the pad column does).

Measured-schedule notes (from NTFF traces of this exact kernel):
- The PE matmul stream runs gap-free at ~2.35GHz effective once HAM
  grants full clock; the stream itself (~31us) is the bf16 roofline.
  The tunable parts are the head (time to first real matmul) and the
  tail (last matmul -> all engines idle; a fixed ~7.5us walrus
  postamble that clears every semaphore follows and cannot be changed).
- Head: the sync HWDGE ring starts moving data ~2.7us into the exec
  window at ~170GB/s; the scalar ring ~3.7us at ~107-170GB/s (2048B
  runs are descriptor-limited).  The first real matmul group is a
  small 5-row chunk needing only w00 + x0 rows 0..6, and its two
  inputs ride the two rings CONCURRENTLY:
    sync:   w00, x0B[0:64], x0C[0:64], w01, x1
    scalar: x0A (rows 0..6), x0B[64:128], x0C[64:128], w(1,*)
  so the first real matmul fires ~6.0us in (vs 7.5 for the naive
  order); every later piece lands just ahead of the group needing it.
- A bf16 matmul warmup on gpsimd-memset garbage bridges the DMA wait
  and absorbs the HAM 1.2GHz cold period (~3.4us); it is sized to end
  just before w00+x0A land.
- Drains fuse bias add, bf16 downcast and parity de-interleave in one
  DVE tensor_scalar_add.  Chunk pairs drain as ONE op reading both
  PSUM banks of a [128,2,512] two-bank tile (4D APs): 16 drains
  instead of 28 -> fewer cross-engine sem edges and a shorter tail.
- Outputs: image-0 assembles in SBUF, leaves as 3 row-band DMAs
  (gpsimd / HW-rings / gpsimd); image-1 is band-major with bands
  (14,11,6,2) parity rows: the big early bands ride gpsimd while the
  HW rings only carry the last two small bands, so little trails the
  final matmul.
"""

import numpy as np
import ml_dtypes

import concourse.bass as bass
import concourse.bacc as bacc
import concourse.tile as tile
from concourse import mybir
from concourse.bass_utils import run_bass_kernel_spmd

N_CORES = 8
B_PER = 2  # images per core

F32 = mybir.dt.float32
BF16 = mybir.dt.bfloat16

PW = 34            # padded x width (32 + 1 left + 1 right)
XLEN = PW * PW     # 1156 padded x elems per partition
XPAD = 1160        # sbuf/dram x free size (AP slack for the last chunk)

# (m0, R) parity-row chunks; rows m0..m0+R-1 of the 33-row parity grid.
# One PSUM accumulation group per chunk: N = R*34 fp32 <= 2KB PSUM bank.
# Image 0 is [single, pairA, pairB] when SINGLE_FIRST else
# [pairA, pairB, single]; the two pair chunks must have equal R.
CHUNKS0 = [(0, 5), (5, 14), (19, 14)]
SINGLE_FIRST = True
# image 1 drains through progressively smaller output bands so the
# last band's DMA is tiny and almost nothing trails the final matmul
CHUNKS1 = [(0, 14), (14, 11), (25, 6), (31, 2)]
PAIR_BUFS = 3
SINGLE_BUFS = 1

# x0 row-pieces (padded-row element ranges of the 1156-elem plane):
# piece A covers the first chunk's rows, B the second chunk's, C the rest.
X0A = 240
X0B = 714
N_WARMUP = 13
# HEAD2: w00 rides the sync ring whole while x0A rides the scalar ring
# whole (the two first-group inputs land concurrently); x0B/x0C are
# partition-halved across both rings.
HEAD2 = True
# HXFUSE: host packs [x0 rows 0..7 | w00] into one DRAM tensor arriving
# as ONE fat-run sync DMA.  A/B-measured WORSE than HEAD2 (serializing
# 385KB onto one ring loses to the two rings working in parallel on
# w00 and x0A); kept for reference, off by default.
HXFUSE = False
X0MAIN = 170
# TAIL3: split the second-to-last image-1 band across sync/scalar/
# gpsimd.  A/B-measured neutral-to-worse (the gpsimd SWDGE piece
# straggles); off by default.
TAIL3 = False

# partition split across the two HW rings for halved transfers
PSPLIT = 64


def _emit_group(nc, ps, wv, xv, m0, R):
    """One PSUM accumulation group: 8 matmuls for one class, one chunk.
    wv is the class's [ci', c, i, j, co] weight view; xv the [ci', c, e]
    padded-x view.  The rhs stays contiguous (the pad column rides along
    and is cropped on drain)."""
    nf = R * PW
    k = 0
    for c in range(2):
        for i in range(2):
            for j in range(2):
                off = (m0 - i + 1) * PW + (1 - j)
                nc.tensor.matmul(
                    ps,
                    wv[:, c, i, j, :],
                    xv[:, c, off:off + nf],
                    start=(k == 0),
                    stop=(k == 7),
                )
                k += 1


TARGET_BIR = False


def build_nc(debug: bool = False) -> bass.Bass:
    nc = bacc.Bacc("TRN2", target_bir_lowering=TARGET_BIR, debug=debug,
                   num_devices=N_CORES)

    # x arrives host-padded bf16: [img, ci'=128, c=2, 34*34+tail] flat
    x_d = nc.declare_dram_parameter("x", [B_PER, 128, 2, XPAD], BF16,
                                    isOutput=False)
    # w layout: [ci'=128, p, q, c, i, j, co] -- a whole class is one
    # contiguous 2KB run per partition, so one DMA per class is efficient
    w_d = nc.declare_dram_parameter("w", [128, 2, 2, 2, 2, 2, 128], BF16,
                                    isOutput=False)
    b_d = nc.declare_dram_parameter("b", [128, 1], F32, isOutput=False)
    # fused [x0 rows 0..7 | w00] head transfer (see HXFUSE)
    hx_d = nc.declare_dram_parameter("hx", [128, 2 * X0A + 1024], BF16,
                                     isOutput=False)
    # y ships as bf16 (host upcasts): halves output HBM traffic and
    # doubles DVE drain throughput; adds ~2e-3 rounding (budget 2e-2)
    y_d = nc.declare_dram_parameter("y", [B_PER, 128, 66, 66], BF16,
                                    isOutput=True)

    with tile.TileContext(nc) as tc:
        with (
            tc.tile_pool(name="wp", bufs=1) as wpool,
            tc.tile_pool(name="bp", bufs=1) as bpool,
            tc.tile_pool(name="xp", bufs=B_PER) as xpool,
            tc.tile_pool(name="yp", bufs=1) as ypool,
            tc.tile_pool(name="ybp", bufs=4) as bandpool,
            tc.tile_pool(name="wu", bufs=1) as wupool,
            tc.tile_pool(name="hxp", bufs=1) as hxpool,
            tc.tile_pool(name="pd", bufs=PAIR_BUFS, space="PSUM") as pairpool,
            tc.tile_pool(name="psg", bufs=SINGLE_BUFS, space="PSUM") as singlepool,
            tc.tile_pool(name="pw", bufs=1, space="PSUM") as warmpool,
        ):
            # PE warm-up on gpsimd-memset garbage (gpsimd runs user code
            # first, so the warmup starts ~1us earlier than via vector).
            # HAM starts the PE at 1.2GHz and unthrottles after ~3.4us of
            # sustained activity; the dummies bridge until w00+x0A land.
            wub = wupool.tile([128, 512], BF16)
            nc.gpsimd.memset(wub[:], 0.0)
            wps = warmpool.tile([128, 512], F32)
            for _ in range(N_WARMUP):
                nc.tensor.matmul(wps[:], wub[:, 0:128], wub[:],
                                 start=True, stop=True)

            # bias rides gpsimd (the output queue, idle at start) so it
            # lands before the first drain without delaying inputs
            bt = bpool.tile([128, 1], F32)
            nc.gpsimd.dma_start(out=bt[:], in_=b_d[:])

            wt = wpool.tile([128, 2, 2, 2, 2, 2, 128], BF16)
            xt = [xpool.tile([128, 2, XPAD], BF16, name=f"x{img}", tag="xt")
                  for img in range(B_PER)]
            hxt = hxpool.tile([128, 2 * X0A + 1024], BF16)
            hx_x = hxt[:, 0:2 * X0A].rearrange("p (c e) -> p c e", c=2)
            hx_w = hxt[:, 2 * X0A:].rearrange("p (c i j o) -> p c i j o",
                                              c=2, i=2, j=2)

            def class_w(p, q):
                if HXFUSE and p == 0 and q == 0:
                    return hx_w
                return wt[:, p, q]

            # Input schedule (see module docstring): the two first-group
            # inputs (x0 piece A and w00) land as early as possible; the
            # later x0 pieces arrive just ahead of the chunks needing them.
            if HXFUSE:
                nc.sync.dma_start(out=hxt[:], in_=hx_d[:])
                nc.sync.dma_start(out=xt[0][0:PSPLIT, :, X0MAIN:X0B],
                                  in_=x_d[0][0:PSPLIT, :, X0MAIN:X0B])
                nc.scalar.dma_start(out=xt[0][PSPLIT:128, :, X0MAIN:X0B],
                                    in_=x_d[0][PSPLIT:128, :, X0MAIN:X0B])
                nc.scalar.dma_start(out=xt[0][:, :, X0B:XPAD],
                                    in_=x_d[0][:, :, X0B:XPAD])
                nc.sync.dma_start(out=wt[:, 0, 1], in_=w_d[:, 0, 1])
                nc.scalar.dma_start(out=wt[:, 1], in_=w_d[:, 1])
                nc.sync.dma_start(out=xt[1][:], in_=x_d[1])
            elif HEAD2:
                nc.scalar.dma_start(out=xt[0][:, :, 0:X0A],
                                    in_=x_d[0][:, :, 0:X0A])
                nc.sync.dma_start(out=wt[:, 0, 0], in_=w_d[:, 0, 0])
                # x0B rides mostly on the scalar ring (idle after x0A)
                # -- its sync share queues behind the 2.4us w00 transfer
                # and was the systematic ~0.4us PE stall at chunk 2
                nc.sync.dma_start(out=xt[0][0:32, :, X0A:X0B],
                                  in_=x_d[0][0:32, :, X0A:X0B])
                nc.scalar.dma_start(out=xt[0][32:128, :, X0A:X0B],
                                    in_=x_d[0][32:128, :, X0A:X0B])
                nc.sync.dma_start(out=xt[0][0:PSPLIT, :, X0B:XPAD],
                                  in_=x_d[0][0:PSPLIT, :, X0B:XPAD])
                nc.scalar.dma_start(out=xt[0][PSPLIT:128, :, X0B:XPAD],
                                    in_=x_d[0][PSPLIT:128, :, X0B:XPAD])
                nc.sync.dma_start(out=wt[:, 0, 1], in_=w_d[:, 0, 1])
                nc.scalar.dma_start(out=wt[:, 1], in_=w_d[:, 1])
                nc.sync.dma_start(out=xt[1][:], in_=x_d[1])
            else:
                nc.sync.dma_start(out=xt[0][:, :, 0:X0A],
                                  in_=x_d[0][:, :, 0:X0A])
                nc.sync.dma_start(out=wt[:, 0, 0], in_=w_d[:, 0, 0])
                nc.scalar.dma_start(out=xt[0][:, :, X0A:X0B],
                                    in_=x_d[0][:, :, X0A:X0B])
                nc.scalar.dma_start(out=xt[0][:, :, X0B:XPAD],
                                    in_=x_d[0][:, :, X0B:XPAD])
                nc.sync.dma_start(out=wt[:, 0, 1], in_=w_d[:, 0, 1])
                nc.scalar.dma_start(out=wt[:, 1], in_=w_d[:, 1])
                nc.sync.dma_start(out=xt[1][:], in_=x_d[1])

            def drain(ps_view, out_view):
                nc.vector.tensor_scalar_add(out_view, ps_view, bt[:])

            # ---- image 0: class-major (w classes stream in one at a
            # time).  Chunks 1+2 (equal R=14) accumulate into the two
            # banks of one pair tile and drain as a single DVE op.
            yt = ypool.tile([128, 66, 66], BF16)
            if SINGLE_FIRST:
                (m0s, Rs), (m0a, Ra), (m0b, Rb) = CHUNKS0
            else:
                (m0a, Ra), (m0b, Rb), (m0s, Rs) = CHUNKS0
            assert Ra == Rb and m0b == m0a + Ra
            for p in range(2):
                for q in range(2):
                    pt = pairpool.tile([128, 2, 512], F32)
                    st = singlepool.tile([128, 512], F32)

                    def do_single():
                        xv = hx_x if HXFUSE else xt[0]
                        _emit_group(nc, st[:, 0:Rs * PW], class_w(p, q), xv,
                                    m0s, Rs)
                        drain(
                            st[:, 0:Rs * PW]
                            .rearrange("p (m n) -> p m n", n=PW)[:, :, 0:33],
                            yt[:, p::2, q::2][:, m0s:m0s + Rs, :],
                        )

                    def do_pair():
                        _emit_group(nc, pt[:, 0, 0:Ra * PW], class_w(p, q),
                                    xt[0], m0a, Ra)
                        _emit_group(nc, pt[:, 1, 0:Rb * PW], class_w(p, q),
                                    xt[0], m0b, Rb)
                        # both pair chunks leave in one 2-bank drain
                        drain(
                            pt[:, :, 0:Ra * PW]
                            .rearrange("p b (m n) -> p b m n",
                                       n=PW)[:, :, :, 0:33],
                            yt[:, p::2, q::2][:, m0a:m0a + Ra + Rb, :]
                            .rearrange("p (b m) n -> p b m n", b=2),
                        )

                    if SINGLE_FIRST:
                        do_single()
                        do_pair()
                    else:
                        do_pair()
                        do_single()
            for bi, (m0, R) in enumerate(CHUNKS0):
                rows = slice(2 * m0, 2 * (m0 + R))
                if bi == 1:
                    nc.sync.dma_start(out=y_d[0][0:PSPLIT, rows],
                                      in_=yt[0:PSPLIT, rows])
                    nc.scalar.dma_start(out=y_d[0][PSPLIT:128, rows],
                                        in_=yt[PSPLIT:128, rows])
                else:
                    nc.gpsimd.dma_start(out=y_d[0][:, rows], in_=yt[:, rows])

            # ---- image 1: band-major.  Per band the four classes fill
            # two pair tiles (p=0 -> banks q=0,1 of tile A; p=1 -> tile
            # B) and drain as two DVE ops.  The first two bands ride
            # gpsimd (fast outbound) while y0 drains the HW rings; the
            # small last bands ride the HW rings, idle again by then.
            for bi, (m0, R) in enumerate(CHUNKS1):
                band = bandpool.tile([128, 2 * R, 66], BF16)
                for p in range(2):
                    pt = pairpool.tile([128, 2, 512], F32)
                    for q in range(2):
                        _emit_group(nc, pt[:, q, 0:R * PW], class_w(p, q),
                                    xt[1], m0, R)
                    drain(
                        pt[:, :, 0:R * PW]
                        .rearrange("p b (m n) -> p b m n", n=PW)[:, :, :, 0:33],
                        band[:, p::2, :]
                        .rearrange("p m (n q) -> p q m n", q=2),
                    )
                y_view = y_d[1][:, 2 * m0:2 * (m0 + R), :]
                if bi < 2:
                    nc.gpsimd.dma_start(out=y_view, in_=band[:])
                elif TAIL3 and bi == 2:
                    # 3-way split so the last band's ring DMAs aren't
                    # queued behind this band's bytes
                    nc.sync.dma_start(out=y_view[0:32], in_=band[0:32])
                    nc.scalar.dma_start(out=y_view[32:64], in_=band[32:64])
                    nc.gpsimd.dma_start(out=y_view[64:128], in_=band[64:128])
                else:
                    nc.sync.dma_start(out=y_view[0:PSPLIT], in_=band[0:PSPLIT])
                    nc.scalar.dma_start(out=y_view[PSPLIT:128],
                                        in_=band[PSPLIT:128])

    nc.compile()
    return nc


_nc_cache = None


def _get_nc():
    global _nc_cache
    if _nc_cache is None:
        _nc_cache = build_nc()
    return _nc_cache


def make_in_maps(x: np.ndarray, weight: np.ndarray, bias: np.ndarray):
    # w[ci,co,kh,kw] -> [ci', p, q, c, i, j, co]
    w7 = (
        np.asarray(weight, dtype=np.float32)
        .reshape(2, 128, 128, 2, 2, 2, 2)      # [c, ci', co, i, p, j, q]
        .transpose(1, 4, 6, 0, 3, 5, 2)        # -> [ci', p, q, c, i, j, co]
    )
    w_host = np.ascontiguousarray(w7.astype(ml_dtypes.bfloat16))
    b_host = np.ascontiguousarray(
        np.asarray(bias, dtype=np.float32).reshape(128, 1)
    )
    x = np.asarray(x, dtype=np.float32)
    # host-side zero-pad into the 34x34(+tail) layout, ci split [c, ci']
    # transposed to [ci', c], bf16
    xpad = np.zeros((16, 2, 128, XPAD), dtype=np.float32)
    xpad[:, :, :, :XLEN].reshape(16, 2, 128, PW, PW)[:, :, :, 1:33, 1:33] = (
        x.reshape(16, 2, 128, 32, 32)
    )
    xpad = np.ascontiguousarray(
        xpad.transpose(0, 2, 1, 3).astype(ml_dtypes.bfloat16)
    )
    w00_flat = w_host[:, 0, 0].reshape(128, 1024)
    return [
        {
            "x": xpad[B_PER * i:B_PER * (i + 1)],
            "w": w_host,
            "b": b_host,
            # fused [x0 rows 0..7 | w00] head transfer (one fat run)
            "hx": np.ascontiguousarray(np.concatenate(
                [xpad[B_PER * i][:, :, 0:X0A].reshape(128, 2 * X0A),
                 w00_flat], axis=1)),
        }
        for i in range(N_CORES)
    ]


def kernel(x: np.ndarray, weight: np.ndarray, bias: np.ndarray) -> np.ndarray:
    nc = _get_nc()
    in_maps = make_in_maps(x, weight, bias)
    res = run_bass_kernel_spmd(nc, in_maps, list(range(N_CORES)))
    out = np.concatenate([r["y"] for r in res.results], axis=0)
    return np.ascontiguousarray(out.astype(np.float32))
